# revision 1
# baseline (speedup 1.0000x reference)
"""Trainium2 Bass kernel for nn_Decoder (2-layer LSTM + 3 FC + top-k beam decode).

Strategy: pure data parallelism over batch (2048 -> 8 cores x 256).
Activations feature-major [feat, batch]. LSTM/fc1/fc2 fp32 on PE.
fc3 (256->10000, padded to 20 full psum banks) as fp16 3-term split
matmul on x256-scaled weights (argmax is scale-invariant; 3-term fp16
keeps ~2^-22 relative accuracy, well under the measured 2.6e-6 decision
margin).  fc1+fc2 fused on host in float64.  Argmax scans run directly
on PSUM per 2048-col group; bias placement is hybrid (PE ones-matmul
pairs for groups 2,4; in-place DVE adds for 0,1,3 - measured optimum).
Next step's lstm1 h-matmuls pre-run into the argmax tail.  Host
assembles the [B,16,4,2] trajectory from per-step argmax indices.
"""
import numpy as np
import ml_dtypes

B, D, H = 2048, 256, 256
K4, QW, QL, DELTA = 4, 100, 100, 16
Q = QW * QL
NCORES = 8
BS = B // NCORES          # 256 rows per core
G = 5                     # fc3 groups per batch-chunk
TW = 512                  # psum tile width (one bank)
QP = 20 * TW              # padded fc3 width (10240); cols >= Q get -6e4 bias
GRP = 4 * TW              # columns per group (2048)
PE_BIAS_GROUPS = (2, 4)   # these groups get fp16 bias matmuls on the PE
                          # and are scanned straight from PSUM; the rest
                          # get the bias added on DVE (engine balancing)

_CACHE = {}


def _build_nc(delta=DELTA):
    import concourse.mybir as mybir
    import concourse.tile as tile
    import concourse.bacc as bacc
    from concourse.masks import make_identity

    F32 = mybir.dt.float32
    F16 = mybir.dt.float16
    U32 = mybir.dt.uint32
    AF = mybir.ActivationFunctionType
    ALU = mybir.AluOpType

    nc = bacc.Bacc(None, target_bir_lowering=False, debug=False)

    def din(name, shape, dt=F32):
        return nc.dram_tensor(name, shape, dt, kind="ExternalInput")

    # state / early inputs
    x_in = din("x_fm", [128, 2, BS])
    h1_in = din("h1_fm", [128, 2, BS])
    c1_in = din("c1_fm", [128, 2, BS])
    h2_in = din("h2_fm", [128, 2, BS])
    c2_in = din("c2_fm", [128, 2, BS])
    b1_in = din("b1r", [128, 8])
    b2_in = din("b2r", [128, 8])
    w1ih_in = din("w1ihT", [128, 2, 4 * H])
    w1hh_in = din("w1hhT", [128, 2, 4 * H])
    w2ih_in = din("w2ihT", [128, 2, 4 * H])
    w2hh_in = din("w2hhT", [128, 2, 4 * H])
    fc21_in = din("fc21T", [128, 2, H])
    fc21b_in = din("fc21br", [128, 2])
    fcqw_in = din("fcqwT", [100, 128])
    fcql_in = din("fcqlT", [100, 128])
    fcqwb_in = din("fcqwb", [128, 1])
    fcqlb_in = din("fcqlb", [128, 1])
    b3rep_in = din("b3rep", [128, QP])
    ones_in = din("ones16", [1, 128], F16)
    b3h_in = din("b3h", [1, QP], F16)
    b3l_in = din("b3l", [1, QP], F16)
    w3h_in = din("w3h", [128, 2, QP], F16)
    w3l_in = din("w3l", [128, 2, QP], F16)

    idx_out = nc.dram_tensor("idx_out", [2, 128, 20], U32, kind="ExternalOutput")

    with tile.TileContext(nc) as tc:
        with (
            tc.tile_pool(name="wp", bufs=1) as wp,
            tc.tile_pool(name="st", bufs=1) as st,
            tc.tile_pool(name="wk", bufs=2) as wk,
            tc.tile_pool(name="ps", bufs=2, space="PSUM") as ps,
        ):
            # ---- loads: states + lstm1 first so step 0 starts early ----
            def wload(src, shape, tag, dt=F32):
                t = wp.tile(shape, dt, tag=tag, name=tag)
                nc.sync.dma_start(t[:], src[:])
                return t

            def sload(src, tag):
                t = st.tile([128, 2, BS], F32, tag=tag, name=tag)
                nc.sync.dma_start(t[:], src[:])
                return t

            emb_t = sload(x_in, "emb")
            h1_t = sload(h1_in, "h1")
            c1_t = sload(c1_in, "c1")
            h2_t = sload(h2_in, "h2")
            c2_t = sload(c2_in, "c2")
            b1r = wload(b1_in, [128, 8], "b1r")
            b2r = wload(b2_in, [128, 8], "b2r")
            w1hh = wload(w1hh_in, [128, 2, 4 * H], "w1hh")
            w1ih = wload(w1ih_in, [128, 2, 4 * H], "w1ih")
            w2hh = wload(w2hh_in, [128, 2, 4 * H], "w2hh")
            w2ih = wload(w2ih_in, [128, 2, 4 * H], "w2ih")
            fc21b = wload(fc21b_in, [128, 2], "fc21b")
            fc21 = wload(fc21_in, [128, 2, H], "fc21")
            fcqw = wload(fcqw_in, [100, 128], "fcqw")
            fcql = wload(fcql_in, [100, 128], "fcql")
            fcqwb = wload(fcqwb_in, [128, 1], "fcqwb")
            fcqlb = wload(fcqlb_in, [128, 1], "fcqlb")
            ones16 = wp.tile([1, 128], F16, name="ones16")
            nc.sync.dma_start(ones16[:], ones_in[:])
            b3h = wp.tile([1, QP], F16, name="b3h")
            nc.sync.dma_start(b3h[:], b3h_in[:])
            b3l = wp.tile([1, QP], F16, name="b3l")
            nc.sync.dma_start(b3l[:], b3l_in[:])
            w3h = wp.tile([128, 2, QP], F16, tag="w3h", name="w3h")
            w3l = wp.tile([128, 2, QP], F16, tag="w3l", name="w3l")
            dve_groups = [g for g in range(G) if g not in PE_BIAS_GROUPS]
            b3rep = wp.tile([128, len(dve_groups) * GRP], F32, tag="b3rep",
                            name="b3rep")
            for _g in range(G):
                _s = slice(_g * GRP, (_g + 1) * GRP)
                nc.sync.dma_start(w3h[:, :, _s], w3h_in[:, :, _s])
                nc.sync.dma_start(w3l[:, :, _s], w3l_in[:, :, _s])
                if _g in dve_groups:
                    _slot = dve_groups.index(_g)
                    nc.sync.dma_start(
                        b3rep[:, _slot * GRP:(_slot + 1) * GRP], b3rep_in[:, _s])

            ident = wp.tile([128, 128], F32)
            make_identity(nc, ident[:])
            io_f = wp.tile([128, 100], F32)
            nc.gpsimd.iota(io_f[:], pattern=[[1, 100]], base=0, channel_multiplier=0,
                           allow_small_or_imprecise_dtypes=True)
            io40 = wp.tile([128, 40], F32)
            nc.gpsimd.iota(io40[:], pattern=[[1, 40]], base=0, channel_multiplier=0,
                           allow_small_or_imprecise_dtypes=True)
            io8 = wp.tile([128, 8], F32)
            nc.gpsimd.iota(io8[:], pattern=[[1, 8]], base=0, channel_multiplier=0,
                           allow_small_or_imprecise_dtypes=True)

            outi = st.tile([128, 2, 20], U32, tag="outi", name="outi")
            nc.vector.memset(outi[:], 0)

            # persistent per-bc scratch for candidates / one-hots
            cand_v = [st.tile([128, 8 * G], F32, tag=f"cv{bc}", name=f"cv{bc}")
                      for bc in range(2)]
            cand_i = [st.tile([128, 8 * G], F32, tag=f"ci{bc}", name=f"ci{bc}")
                      for bc in range(2)]
            # t>=1: per-group max (padded to 8) and winning index
            gmax8 = [st.tile([128, 8], F32, tag=f"gm{bc}", name=f"gm{bc}")
                     for bc in range(2)]
            gq = [st.tile([128, 8], F32, tag=f"gq{bc}", name=f"gq{bc}")
                  for bc in range(2)]
            ohw_b = [st.tile([128, 100], F32, tag=f"ohw{bc}", name=f"ohw{bc}")
                     for bc in range(2)]
            ohl_b = [st.tile([128, 100], F32, tag=f"ohl{bc}", name=f"ohl{bc}")
                     for bc in range(2)]
            ohwT = st.tile([100, 256], F32, tag="ohwT", name="ohwT")
            ohlT = st.tile([100, 256], F32, tag="ohlT", name="ohlT")
            for bc in range(2):
                nc.vector.memset(gmax8[bc][:], -3.0e38)
                nc.vector.memset(gq[bc][:], 0.0)

            def pbig():
                return ps.tile([128, 2048], F32, tag="big", name="big")

            def lstm_mms_h(gpt, hT, whh):
                # start=True only on the first matmul touching each psum
                # bank (2 gates share a bank): a later start would clear the
                # bank's has_written bits and the deferred wih matmuls would
                # overwrite instead of accumulate.
                for g in range(8):
                    sl = slice(128 * g, 128 * (g + 1))
                    nc.tensor.matmul(gpt[:, 256 * g:256 * (g + 1)],
                                     whh[:, 0, sl], hT[:, 0, :],
                                     start=(g % 2 == 0), stop=False)
                    nc.tensor.matmul(gpt[:, 256 * g:256 * (g + 1)],
                                     whh[:, 1, sl], hT[:, 1, :],
                                     start=False, stop=False)

            def lstm_finish(gpt, inp, hT, cT, wih, br):
                def gsl(g):
                    return gpt[:, 256 * g:256 * (g + 1)]

                for g in range(8):
                    sl = slice(128 * g, 128 * (g + 1))
                    nc.tensor.matmul(gsl(g), wih[:, 0, sl], inp[:, 0, :],
                                     start=False, stop=False)
                for g in range(8):
                    sl = slice(128 * g, 128 * (g + 1))
                    nc.tensor.matmul(gsl(g), wih[:, 1, sl], inp[:, 1, :],
                                     start=False, stop=True)
                for ch in range(2):
                    si = wk.tile([128, 256], F32, tag="si", bufs=1)
                    sf = wk.tile([128, 256], F32, tag="sf", bufs=1)
                    tg = wk.tile([128, 256], F32, tag="tg", bufs=1)
                    so = wk.tile([128, 256], F32, tag="so", bufs=1)
                    nc.scalar.activation(si[:], gsl(0 + ch), AF.Sigmoid,
                                         bias=br[:, 0 + ch:1 + ch])
                    nc.scalar.activation(sf[:], gsl(2 + ch), AF.Sigmoid,
                                         bias=br[:, 2 + ch:3 + ch])
                    nc.scalar.activation(tg[:], gsl(4 + ch), AF.Tanh,
                                         bias=br[:, 4 + ch:5 + ch])
                    nc.scalar.activation(so[:], gsl(6 + ch), AF.Sigmoid,
                                         bias=br[:, 6 + ch:7 + ch])
                    t1 = wk.tile([128, 256], F32, tag="t1", bufs=1)
                    t2 = wk.tile([128, 256], F32, tag="t2", bufs=1)
                    nc.vector.tensor_mul(t1[:], sf[:], cT[:, ch, :])
                    nc.vector.tensor_mul(t2[:], si[:], tg[:])
                    nc.vector.tensor_add(cT[:, ch, :], t1[:], t2[:])
                    t3 = wk.tile([128, 256], F32, tag="t3", bufs=1)
                    nc.scalar.activation(t3[:], cT[:, ch, :], AF.Tanh)
                    nc.vector.tensor_mul(hT[:, ch, :], so[:], t3[:])

            def lstm_layer(inp, hT, cT, wih, whh, br):
                gpt = pbig()
                lstm_mms_h(gpt, hT, whh)
                lstm_finish(gpt, inp, hT, cT, wih, br)

            def fc3_group(grp, y2h, y2l, bsl):
                """matmuls for one 2000-col group; returns the psum tile."""
                gp = pbig()
                n0 = grp * GRP

                def tslice(tt):
                    return gp[:, tt * TW:(tt + 1) * TW]

                def wsl(w, k, tt):
                    return w[:, k, n0 + tt * TW:n0 + (tt + 1) * TW]

                # lhsT-reuse sweeps; 'start' on first touch, 'stop' on last
                for tt in range(4):
                    nc.tensor.matmul(tslice(tt), y2h[:, 0, bsl], wsl(w3h, 0, tt),
                                     start=True, stop=False)
                for tt in range(4):
                    nc.tensor.matmul(tslice(tt), y2h[:, 0, bsl], wsl(w3l, 0, tt),
                                     start=False, stop=False)
                for tt in range(4):
                    nc.tensor.matmul(tslice(tt), y2h[:, 1, bsl], wsl(w3h, 1, tt),
                                     start=False, stop=False)
                for tt in range(4):
                    nc.tensor.matmul(tslice(tt), y2h[:, 1, bsl], wsl(w3l, 1, tt),
                                     start=False, stop=False)
                for tt in range(4):
                    nc.tensor.matmul(tslice(tt), y2l[:, 0, bsl], wsl(w3h, 0, tt),
                                     start=False, stop=False)
                pe_bias = grp in PE_BIAS_GROUPS
                for tt in range(4):
                    nc.tensor.matmul(tslice(tt), y2l[:, 1, bsl], wsl(w3h, 1, tt),
                                     start=False, stop=not pe_bias)
                if pe_bias:
                    for tt in range(4):
                        nc.tensor.matmul(tslice(tt), ones16[:],
                                         b3h[:, n0 + tt * TW:n0 + (tt + 1) * TW],
                                         start=False, stop=False)
                    for tt in range(4):
                        nc.tensor.matmul(tslice(tt), ones16[:],
                                         b3l[:, n0 + tt * TW:n0 + (tt + 1) * TW],
                                         start=False, stop=True)
                return gp

            def scan_group(grp, gp, bc, t):
                """Act evacuates psum -> sbuf, Pool adds the bias in place,
                DVE does max8/find_index8 only."""
                n0 = grp * GRP
                if grp in PE_BIAS_GROUPS:
                    lg = gp          # bias already in psum; scan it directly
                else:
                    # add the bias in place in PSUM (no SBUF evacuation)
                    slot = dve_groups.index(grp)
                    nc.vector.tensor_add(gp[:, :], gp[:, :],
                                         b3rep[:, slot * GRP:(slot + 1) * GRP])
                    lg = gp
                if t == 0:
                    sl8 = slice(8 * grp, 8 * grp + 8)
                    m8 = wk.tile([128, 8], F32, tag="m8", name="m8")
                    nc.vector.max(m8[:], lg[:])
                    nc.vector.tensor_copy(cand_v[bc][:, sl8], m8[:])
                    i8 = wk.tile([128, 8], U32, tag="i8", name="i8")
                    nc.vector.max_index(i8[:], m8[:], lg[:])
                    i8f = wk.tile([128, 8], F32, tag="i8f", name="i8f")
                    nc.vector.tensor_copy(i8f[:], i8[:])
                    nc.vector.tensor_scalar(cand_i[bc][:, sl8], i8f[:],
                                            float(GRP * grp), None, op0=ALU.add)
                else:
                    nc.vector.tensor_reduce(gmax8[bc][:, grp:grp + 1], lg[:],
                                            axis=mybir.AxisListType.X,
                                            op=ALU.max)
                    m8 = wk.tile([128, 8], F32, tag="m8", name="m8")
                    nc.vector.tensor_copy(
                        m8[:], gmax8[bc][:, grp:grp + 1].broadcast_to([128, 8]))
                    i8 = wk.tile([128, 8], U32, tag="i8", name="i8")
                    nc.vector.max_index(i8[:], m8[:], lg[:])
                    i8f = wk.tile([128, 1], F32, tag="i8f1", name="i8f")
                    nc.vector.tensor_copy(i8f[:], i8[:, 0:1])
                    nc.vector.tensor_scalar(gq[bc][:, grp:grp + 1],
                                            i8f[:], float(GRP * grp), None,
                                            op0=ALU.add)

            def merge_onehot(bc, t):
                """DVE: merge candidates -> q; write outi; build one-hots."""
                qsel = wk.tile([128, 4], F32, tag="qsel", name="qsel")
                if t == 0:
                    vm8 = wk.tile([128, 8], F32, tag="vm8", name="vm8")
                    nc.vector.max(vm8[:], cand_v[bc][:])
                    pm8 = wk.tile([128, 8], U32, tag="pm8", name="pm8")
                    nc.vector.max_index(pm8[:], vm8[:], cand_v[bc][:])
                    pmf = wk.tile([128, 8], F32, tag="pmf", name="pmf")
                    nc.vector.tensor_copy(pmf[:], pm8[:])
                    for kk in range(4):
                        ohp = wk.tile([128, 40], F32, tag="ohp", name="ohp")
                        nc.vector.tensor_scalar(ohp[:], io40[:], pmf[:, kk:kk + 1],
                                                None, op0=ALU.is_equal)
                        tmq = wk.tile([128, 40], F32, tag="tmq", name="tmq")
                        nc.vector.tensor_mul(tmq[:], ohp[:], cand_i[bc][:])
                        nc.vector.tensor_reduce(qsel[:, kk:kk + 1], tmq[:],
                                                axis=mybir.AxisListType.X,
                                                op=ALU.add)
                else:
                    vm8 = wk.tile([128, 8], F32, tag="vm8", name="vm8")
                    nc.vector.max(vm8[:], gmax8[bc][:])
                    pm8 = wk.tile([128, 8], U32, tag="pm8", name="pm8")
                    nc.vector.max_index(pm8[:], vm8[:], gmax8[bc][:])
                    pmf = wk.tile([128, 8], F32, tag="pmf", name="pmf")
                    nc.vector.tensor_copy(pmf[:], pm8[:])
                    ohp = wk.tile([128, 8], F32, tag="ohp8", name="ohp8")
                    nc.vector.tensor_scalar(ohp[:], io8[:], pmf[:, 0:1],
                                            None, op0=ALU.is_equal)
                    tmq = wk.tile([128, 8], F32, tag="tmq8", name="tmq8")
                    nc.vector.tensor_mul(tmq[:], ohp[:], gq[bc][:])
                    nc.vector.tensor_reduce(qsel[:, 0:1], tmq[:],
                                            axis=mybir.AxisListType.X,
                                            op=ALU.add)
                if t == 0:
                    nc.vector.tensor_copy(outi[:, bc, 0:4], qsel[:, 0:4])
                else:
                    nc.vector.tensor_copy(outi[:, bc, 4 + t - 1:5 + t - 1],
                                          qsel[:, 0:1])
                if t == delta - 1:
                    return
                # fw = floor(q/100) via round-to-nearest cast of q*0.01-0.495
                tq = wk.tile([128, 1], F32, tag="tq", name="tq")
                nc.vector.tensor_scalar(tq[:], qsel[:, 0:1], 0.01, -0.495,
                                        op0=ALU.mult, op1=ALU.add)
                tu = wk.tile([128, 1], U32, tag="tu", name="tu")
                nc.vector.tensor_copy(tu[:], tq[:])
                fwf = wk.tile([128, 1], F32, tag="fwf", name="fwf")
                nc.vector.tensor_copy(fwf[:], tu[:])
                flf = wk.tile([128, 1], F32, tag="flf", name="flf")
                nc.vector.tensor_scalar(flf[:], fwf[:], -100.0, qsel[:, 0:1],
                                        op0=ALU.mult, op1=ALU.add)
                nc.vector.tensor_scalar(ohw_b[bc][:], io_f[:], fwf[:], None,
                                        op0=ALU.is_equal)
                nc.vector.tensor_scalar(ohl_b[bc][:], io_f[:], flf[:], None,
                                        op0=ALU.is_equal)

            def transpose_embed():
                """PE transposes both bc one-hots, Act evacuates, then the
                embed matmuls reuse banks 0/1 of the same psum tile."""
                ptr = pbig()
                for bc in range(2):
                    bsl = slice(128 * bc, 128 * (bc + 1))
                    pw = ptr[0:100, 1024 * bc:1024 * bc + 128]
                    nc.tensor.transpose(pw, ohw_b[bc][:], ident[:])
                    nc.scalar.copy(ohwT[:, bsl], pw)
                    pl = ptr[0:100, 1024 * bc + 512:1024 * bc + 640]
                    nc.tensor.transpose(pl, ohl_b[bc][:], ident[:])
                    nc.scalar.copy(ohlT[:, bsl], pl)
                pe0 = ptr[:, 0:BS]
                pe1 = ptr[:, 512:512 + BS]
                nc.tensor.matmul(pe0, fcqw[:], ohwT[:], start=True, stop=True)
                nc.tensor.matmul(pe1, fcql[:], ohlT[:], start=True, stop=True)
                nc.scalar.activation(emb_t[:, 0, :], pe0, AF.Identity,
                                     bias=fcqwb[:])
                nc.scalar.activation(emb_t[:, 1, :], pe1, AF.Identity,
                                     bias=fcqlb[:])

            gpt1_pend = [None]

            for t in range(delta):
                if gpt1_pend[0] is None:
                    lstm_layer(emb_t, h1_t, c1_t, w1ih, w1hh, b1r)
                else:
                    lstm_finish(gpt1_pend[0], emb_t, h1_t, c1_t, w1ih, b1r)
                    gpt1_pend[0] = None
                lstm_layer(h1_t, h2_t, c2_t, w2ih, w2hh, b2r)

                # fused fc2(fc1(.)) in one matmul (W21 precomputed on host)
                y2 = st.tile([128, 2, BS], F32, tag="y2")
                fpt = pbig()
                for m in range(2):
                    fsl = fpt[:, 512 * m:512 * m + 256]
                    sl = slice(128 * m, 128 * (m + 1))
                    nc.tensor.matmul(fsl, fc21[:, 0, sl], h2_t[:, 0, :],
                                     start=True, stop=False)
                for m in range(2):
                    fsl = fpt[:, 512 * m:512 * m + 256]
                    sl = slice(128 * m, 128 * (m + 1))
                    nc.tensor.matmul(fsl, fc21[:, 1, sl], h2_t[:, 1, :],
                                     start=False, stop=True)
                for m in range(2):
                    fsl = fpt[:, 512 * m:512 * m + 256]
                    nc.scalar.activation(y2[:, m, :], fsl, AF.Identity,
                                         bias=fc21b[:, m:m + 1])

                # y2 -> fp16 hi/lo split (DVE)
                y2h = st.tile([128, 2, BS], F16, tag="y2h")
                y2l = st.tile([128, 2, BS], F16, tag="y2l")
                for ch in range(2):
                    nc.vector.tensor_copy(y2h[:, ch, :], y2[:, ch, :])
                for ch in range(2):
                    nc.vector.tensor_sub(y2l[:, ch, :], y2[:, ch, :],
                                         y2h[:, ch, :])

                # fc3 both batch-chunks; scans trail on DVE
                for bc in range(2):
                    bsl = slice(128 * bc, 128 * (bc + 1))
                    for grp in range(G):
                        gp = fc3_group(grp, y2h, y2l, bsl)
                        scan_group(grp, gp, bc, t)
                    merge_onehot(bc, t)

                if t == delta - 1:
                    continue
                # pre-run next step's lstm1 h-matmuls into the tail idle
                gpt1_pend[0] = pbig()
                lstm_mms_h(gpt1_pend[0], h1_t, w1hh)
                transpose_embed()

            for bc in range(2):
                nc.sync.dma_start(idx_out[bc], outi[:, bc, :])
    nc.finalize()
    return nc


def _prep_shared(inputs):
    f32 = np.float32
    f16 = ml_dtypes.float16 if hasattr(ml_dtypes, "float16") else np.float16

    def fm(w):  # [out,in] -> lhsT layout [128,2,out] (partition-major)
        wt = np.ascontiguousarray(w.T.astype(f32))        # [in, out]
        return np.ascontiguousarray(
            wt.reshape(2, 128, wt.shape[1]).transpose(1, 0, 2))

    # fc1+fc2 fused in float64 on host: y2 = h2 @ (fc2@fc1).T + (fc2@b1+b2)
    W21 = (inputs["fc2_W"].astype(np.float64)
           @ inputs["fc1_W"].astype(np.float64)).astype(f32)
    b21 = (inputs["fc2_W"].astype(np.float64)
           @ inputs["fc1_b"].astype(np.float64)
           + inputs["fc2_b"].astype(np.float64)).astype(f32)

    # fc3 scaled x256 (argmax invariant), fp16 hi/lo split, padded to QP
    # with -6e4 bias so padded cols never win the argmax
    QPAD = 20 * 512
    fc3T = np.zeros((256, QPAD), f32)
    fc3T[:, :Q] = inputs["fc3_W"].T.astype(f32) * 256.0
    w3h = fc3T.astype(np.float16)
    w3l = (fc3T - w3h.astype(f32)).astype(np.float16)
    b3s = np.full((QPAD,), -60000.0, f32)
    b3s[:Q] = inputs["fc3_b"].astype(f32) * 256.0
    b3rep = np.ascontiguousarray(np.broadcast_to(b3s, (128, QPAD)))
    b3h = b3s.astype(np.float16)
    b3l = (b3s - b3h.astype(f32)).astype(np.float16)

    shared = {
        "w1ihT": fm(inputs["lstm1_Wih"]),
        "w1hhT": fm(inputs["lstm1_Whh"]),
        "w2ihT": fm(inputs["lstm2_Wih"]),
        "w2hhT": fm(inputs["lstm2_Whh"]),
        "fc21T": fm(W21),
        "w3h": np.ascontiguousarray(w3h.reshape(2, 128, QPAD).transpose(1, 0, 2)),
        "w3l": np.ascontiguousarray(w3l.reshape(2, 128, QPAD).transpose(1, 0, 2)),
        "b3rep": b3rep,
        "b3h": b3h.reshape(1, QPAD),
        "b3l": b3l.reshape(1, QPAD),
        "ones16": np.ones((1, 128), np.float16),
        "fcqwT": np.ascontiguousarray(inputs["fcqw_W"].T.astype(f32)),
        "fcqlT": np.ascontiguousarray(inputs["fcql_W"].T.astype(f32)),
        "b1r": inputs["lstm1_b"].astype(f32).reshape(8, 128).T.copy(),
        "b2r": inputs["lstm2_b"].astype(f32).reshape(8, 128).T.copy(),
        "fc21br": b21.astype(f32).reshape(2, 128).T.copy(),
        "fcqwb": inputs["fcqw_b"].astype(f32).reshape(128, 1),
        "fcqlb": inputs["fcql_b"].astype(f32).reshape(128, 1),
    }
    return shared


def _per_core(inputs, c):
    f32 = np.float32
    sl = slice(c * BS, (c + 1) * BS)

    def fmT(a):  # [BS, 256] -> [128, 2, BS] (partition-major)
        t = np.ascontiguousarray(a.T.astype(f32)).reshape(2, 128, BS)
        return np.ascontiguousarray(t.transpose(1, 0, 2))

    return {
        "x_fm": fmT(inputs["x"][sl, 0, :]),
        "h1_fm": fmT(inputs["h1"][0, sl]),
        "c1_fm": fmT(inputs["c1"][0, sl]),
        "h2_fm": fmT(inputs["h2"][0, sl]),
        "c2_fm": fmT(inputs["c2"][0, sl]),
    }


def kernel(**inputs):
    key = "nc"
    if key not in _CACHE:
        _CACHE[key] = _build_nc()
    nc = _CACHE[key]

    shared = _prep_shared(inputs)
    in_maps = []
    for c in range(NCORES):
        m = dict(shared)
        m.update(_per_core(inputs, c))
        in_maps.append(m)

    from concourse.bass_utils import run_bass_kernel_spmd
    res = run_bass_kernel_spmd(nc, in_maps, list(range(NCORES)))
    return assemble(res.results)


def assemble(results):
    traj = np.zeros((B, DELTA, K4, 2), np.float32)
    for c, r in enumerate(results):
        idx = r["idx_out"].reshape(2, 128, 20).astype(np.int64)
        for bc in range(2):
            rows = slice(c * BS + bc * 128, c * BS + (bc + 1) * 128)
            top4 = idx[bc, :, 0:4]
            traj[rows, 0, :, 0] = (top4 % QL).astype(np.float32)
            traj[rows, 0, :, 1] = (top4 // QL).astype(np.float32)
            greedy = idx[bc, :, 4:4 + DELTA - 1]
            traj[rows, 1:, 0, 0] = (greedy % QL).astype(np.float32)
            traj[rows, 1:, 0, 1] = (greedy // QL).astype(np.float32)
    return traj

